# revision 1
# baseline (speedup 1.0000x reference)
"""Trainium2 Bass kernel for the AESUELOGIT segment-reduce problem.

Strategy (8 NeuronCores, SPMD):
  - Shard the 20000 paths across cores ALIGNED TO OD BOUNDARIES (core i owns
    ods [500i, 500(i+1)) and all their paths).  The segmented softmax is then
    fully core-local: no denominator collective is needed.
  - The whole middle of the kernel is ONE software pipeline over 512-path
    blocks: matmul1 (bf16, V split hi/lo) accumulates a block's utilities,
    then per 128-path chunk: transpose -> exp -> segment-sum (bf16 matmul
    over chunk-local od slots) -> denominator gather-back (matmuls that also
    stitch ods straddling chunk boundaries) -> path flows -> matmul2
    accumulation (bf16).  Later blocks' matmuls overlap earlier chunks'
    softmax chain.
  - One ReduceScatter of the (96, 2048) partial link flows; each core runs
    the BPR epilogue on its own 12 day-hour rows in a folded (96, 256)
    layout (no transposes), and the host concatenates the 8 slices.
  - DMA is spread across the two HWDGE rings (sync/scalar) + SWDGE (gpsimd)
    so the D / D^T / constant streams don't serialize on one FIFO.
"""

import os

import numpy as np
import ml_dtypes

import concourse.bacc as bacc
import concourse.bass as bass
import concourse.mybir as mybir
import concourse.tile as tile
from concourse.bass_utils import run_bass_kernel_spmd

F32 = mybir.dt.float32
BF16 = mybir.dt.bfloat16
AF = mybir.ActivationFunctionType
ALU = mybir.AluOpType

ND, NH, NL, NF = 4, 24, 2000, 4
NP, NOD, NCORES = 20000, 4000, 8
DH = ND * NH            # 96
DHS = DH // NCORES      # 12 day-hour rows per core after ReduceScatter
L_PAD = 2048            # links padded to 16*128
KL = L_PAD // 128       # 16 link chunks
FB = L_PAD // 256       # 8 link blocks in the folded epilogue layout
EPS = 1e-12


def _build_program(PSHARD, SLOT):
    """Emit the SPMD Bass program (identical on all cores)."""
    NCH = PSHARD // 128
    NPS = (PSHARD + 511) // 512          # 512-path blocks
    SL1 = SLOT + 1                       # seg matmul emits asc slots + last-od
    nc = bacc.Bacc("TRN2", target_bir_lowering=False, debug=False,
                   num_devices=NCORES)

    # ---- parameters (per-core shards) ----
    p_ft = nc.dram_tensor("ft", [DH, NF * L_PAD], F32, kind="ExternalInput")
    # D link-chunk-major without block padding:
    # dkb[p, PSHARD*k + j] = D[128k+p, j]
    p_d = nc.dram_tensor("dkb", [128, KL * PSHARD], BF16,
                         kind="ExternalInput")
    p_dt = nc.dram_tensor("dtk", [NCH, 128, L_PAD], BF16, kind="ExternalInput")
    p_s = nc.dram_tensor("seg", [128, NCH * SL1], BF16, kind="ExternalInput")
    p_stm = nc.dram_tensor("stm", [SLOT, NCH * 128], BF16,
                           kind="ExternalInput")
    p_stf = nc.dram_tensor("stf", [1, NCH * 128], BF16, kind="ExternalInput")
    p_stb = nc.dram_tensor("stb", [1, NCH * 128], BF16, kind="ExternalInput")
    p_qs = nc.dram_tensor("qsp", [128, NCH], F32, kind="ExternalInput")
    p_th = nc.dram_tensor("th", [1, NF], F32, kind="ExternalInput")
    p_tl = nc.dram_tensor("tl", [128, KL], F32, kind="ExternalInput")
    p_id = nc.dram_tensor("idn", [128, 128], F32, kind="ExternalInput")
    # folded (96, 256) epilogue tensors (host-replicated layouts, no math)
    p_kb = nc.dram_tensor("kb96", [DH, 256], F32, kind="ExternalInput")
    p_bb = nc.dram_tensor("bb96", [DH, 256], F32, kind="ExternalInput")
    p_lab = nc.dram_tensor("lab96", [DH, 256], F32, kind="ExternalInput")
    p_ttf = nc.dram_tensor("ttf96", [DH, 256], F32, kind="ExternalInput")
    p_out = nc.dram_tensor("out", [DH, 256], F32, kind="ExternalOutput")

    with tile.TileContext(nc) as tc:
        with tc.tile_pool(name="const", bufs=1) as cpool, \
             tc.tile_pool(name="dram", bufs=1, space="DRAM") as dpool, \
             tc.tile_pool(name="big", bufs=1) as bpool, \
             tc.tile_pool(name="stream", bufs=4) as spool, \
             tc.tile_pool(name="dkp", bufs=2) as dkpool, \
             tc.tile_pool(name="dtp", bufs=4) as dtpool:

            # early-critical loads on the SP HWDGE ring (features first:
            # they gate V0 -> lhsT -> matmul1)
            fts = bpool.tile([DH, NF * L_PAD], F32, tag="fts")
            for f in range(NF):
                eng = nc.sync if f % 2 == 0 else nc.scalar
                eng.dma_start(fts[:, f * L_PAD:(f + 1) * L_PAD],
                              p_ft.ap()[:, f * L_PAD:(f + 1) * L_PAD])
            ident = cpool.tile([128, 128], F32, tag="ident")
            nc.sync.dma_start(ident[:], p_id.ap())
            tl_sb = cpool.tile([128, KL], F32, tag="tl")
            nc.sync.dma_start(tl_sb[:], p_tl.ap())
            th_sb = cpool.tile([1, NF], F32, tag="th")
            nc.scalar.dma_start(th_sb[:], p_th.ap())

            # later-needed loads on the ACT HWDGE ring / SWDGE
            qs_sb = cpool.tile([128, NCH], F32, tag="qs")
            nc.scalar.dma_start(qs_sb[:], p_qs.ap())
            s_all = cpool.tile([128, NCH * SL1], BF16, tag="s_all")
            nc.scalar.dma_start(s_all[:], p_s.ap())
            stm_all = cpool.tile([SLOT, NCH * 128], BF16, tag="stm_all")
            nc.scalar.dma_start(stm_all[:], p_stm.ap())
            stf_all = cpool.tile([1, NCH * 128], BF16, tag="stf_all")
            nc.scalar.dma_start(stf_all[:], p_stf.ap())
            stb_all = cpool.tile([1, NCH * 128], BF16, tag="stb_all")
            nc.scalar.dma_start(stb_all[:], p_stb.ap())
            kb = cpool.tile([DH, 256], F32, tag="kb")
            nc.scalar.dma_start(kb[:], p_kb.ap())
            bb = cpool.tile([DH, 256], F32, tag="bb")
            nc.scalar.dma_start(bb[:], p_bb.ap())
            lab = cpool.tile([DH, 256], F32, tag="lab")
            nc.scalar.dma_start(lab[:], p_lab.ap())
            ttf = cpool.tile([DH, 256], F32, tag="ttf")
            nc.scalar.dma_start(ttf[:], p_ttf.ap())

            thc = cpool.tile([1, NF], F32, tag="thc")
            nc.vector.tensor_scalar_min(thc[:], th_sb[:], 0.0)
            ones = cpool.tile([1, 128], F32, tag="ones")
            nc.vector.memset(ones[:], 1.0)

            with tc.tile_pool(name="psA", bufs=3, space="PSUM") as psA:
                thb_ps = psA.tile([128, NF], F32, tag="m")
                nc.tensor.matmul(thb_ps[:], ones[:], thc[:],
                                 start=True, stop=True)
                thb = cpool.tile([128, NF], F32, tag="thb")
                nc.scalar.copy(thb[:], thb_ps[:])

                # V0[dh, l] = sum_f theta_f * feats[f], two independent
                # half-link chains so the first transposes start sooner
                v0 = bpool.tile([DH, L_PAD], F32, tag="v0")
                HP = L_PAD // 2
                for h in range(2):
                    sl = slice(h * HP, (h + 1) * HP)
                    nc.vector.tensor_scalar_mul(v0[:, sl], fts[:, sl],
                                                thb[:DH, 0:1])
                    for f in range(1, NF):
                        nc.vector.scalar_tensor_tensor(
                            v0[:, sl],
                            fts[:, f * L_PAD + h * HP:f * L_PAD + (h + 1) * HP],
                            thb[:DH, f:f + 1], v0[:, sl], ALU.mult, ALU.add)

                # lhsT prep (bf16 hi/lo), per link-chunk for pipelining
                lk_all = bpool.tile([128, 128 * KL], F32, tag="lk_all")
                nc.vector.memset(lk_all[:], 0.0)
                lh_all = bpool.tile([128, 128 * KL], BF16, tag="lh_all")
                ll_all = bpool.tile([128, 128 * KL], BF16, tag="ll_all")
                for k in range(KL):
                    vt_ps = psA.tile([128, DH], F32, tag="m")
                    nc.tensor.matmul(vt_ps[:], v0[:, 128 * k:128 * (k + 1)],
                                     ident[:DH, :DH], is_transpose=True,
                                     start=True, stop=True)
                    ksl = slice(128 * k, 128 * k + 128)
                    nc.scalar.copy(lk_all[:, 128 * k:128 * k + DH], vt_ps[:])
                    nc.vector.tensor_copy(
                        lk_all[:, 128 * k + DH:128 * k + DH + 1],
                        tl_sb[:, k:k + 1])
                    nc.vector.tensor_copy(lh_all[:, ksl], lk_all[:, ksl])
                    lup = spool.tile([128, 128], F32, tag="lup")
                    nc.vector.tensor_copy(lup[:], lh_all[:, ksl])
                    nc.vector.tensor_sub(lup[:], lk_all[:, ksl], lup[:])
                    nc.vector.tensor_copy(ll_all[:, ksl], lup[:])

                # ---- the block pipeline ----
                ysb = bpool.tile([97, PSHARD], F32, tag="ysb")
                evt = bpool.tile([128, DH * NCH], F32, tag="evt")
                evb = bpool.tile([128, DH * NCH], BF16, tag="evb")
                tall = bpool.tile([SL1, DH * NCH], BF16, tag="tall")
                ft_bf = bpool.tile([128, 128 * NCH], BF16, tag="ftb")
                nc.vector.memset(ft_bf[:], 0.0)
                qsq = cpool.tile([128, NCH], F32, tag="qsq")
                nc.vector.tensor_mul(qsq[:], qs_sb[:], qs_sb[:])
                SPLIT = (NCH + 1) // 2
                ar_in_a = dpool.tile([DH, L_PAD], F32, tag="arina")
                ar_out_a = dpool.tile([DHS, L_PAD], F32, tag="arouta")
                ar_in_b = dpool.tile([DH, L_PAD], F32, tag="arinb")
                ar_out_b = dpool.tile([DHS, L_PAD], F32, tag="aroutb")

                def chunk_softmax(c):
                    """transpose -> exp -> segment sums for path chunk c."""
                    yt_ps = psA.tile([128, 97], F32, tag="m",
                                     name=f"yt{c}")
                    nc.tensor.matmul(yt_ps[:], ysb[:, 128 * c:128 * (c + 1)],
                                     ident[:97, :97], is_transpose=True,
                                     start=True, stop=True)
                    cvec = spool.tile([128, 1], F32, tag="cvec")
                    nc.scalar.copy(cvec[:], yt_ps[:, DH:DH + 1])
                    nc.scalar.activation(evt[:, DH * c:DH * (c + 1)],
                                         yt_ps[:, 0:DH], AF.Exp, bias=cvec[:])
                    nc.vector.tensor_copy(evb[:, DH * c:DH * (c + 1)],
                                          evt[:, DH * c:DH * (c + 1)])
                    ts_ps = psA.tile([SL1, DH], F32, tag="m",
                                     name=f"seg{c}")
                    nc.tensor.matmul(ts_ps[:],
                                     s_all[:, SL1 * c:SL1 * (c + 1)],
                                     evb[:, DH * c:DH * (c + 1)],
                                     start=True, stop=True)
                    nc.vector.tensor_copy(tall[:, DH * c:DH * (c + 1)],
                                          ts_ps[:])

                def chunk_flow(c):
                    """denominator gather + path flows + matmul2 for chunk c
                    (requires chunk c+1's segment sums, except the last)."""
                    g_ps = psA.tile([128, DH], F32, tag="m", name=f"g{c}")
                    cn = (c + 1) % NCH
                    cp = (c - 1) % NCH
                    nc.tensor.matmul(g_ps[:],
                                     stm_all[:, 128 * c:128 * (c + 1)],
                                     tall[0:SLOT, DH * c:DH * (c + 1)],
                                     start=True, stop=False)
                    nc.tensor.matmul(g_ps[:],
                                     stf_all[:, 128 * c:128 * (c + 1)],
                                     tall[0:1, DH * cn:DH * cn + DH],
                                     start=False, stop=False)
                    # row SLOT of chunk cp = its last-od partial; bounce it
                    # through a base-0 tile so the matmul base partitions match
                    tl_b = spool.tile([1, DH], BF16, tag="tlb")
                    nc.scalar.copy(tl_b[:],
                                   tall[SLOT:SL1, DH * cp:DH * cp + DH])
                    nc.tensor.matmul(g_ps[:],
                                     stb_all[:, 128 * c:128 * (c + 1)],
                                     tl_b[:], start=False, stop=True)
                    rec = spool.tile([128, DH], F32, tag="rec")
                    nc.vector.tensor_scalar_max(rec[:], g_ps[:], 1e-30)
                    nc.vector.reciprocal(rec[:], rec[:])
                    tmp = spool.tile([128, DH], F32, tag="tmp")
                    nc.vector.scalar_tensor_tensor(
                        tmp[:], evt[:, DH * c:DH * (c + 1)],
                        qsq[:, c:c + 1], rec[:], ALU.mult, ALU.mult)
                    nc.vector.tensor_copy(ft_bf[:, 128 * c:128 * c + DH],
                                          tmp[:])
                    dt_t = dtpool.tile([128, L_PAD], BF16, tag="dt",
                                       name=f"dt{c}")
                    # second half loads D^T on the ACT HWDGE ring so they
                    # can't queue behind the in-flight collective on gpsimd
                    dt_eng = nc.gpsimd if c < SPLIT else nc.scalar
                    dt_eng.dma_start(dt_t[:], p_dt.ap()[c])
                    for n in range(L_PAD // 512):
                        nc.tensor.matmul(
                            x_ps[n][:], ft_bf[:, 128 * c:128 * (c + 1)],
                            dt_t[:, 512 * n:512 * (n + 1)],
                            start=(c in (0, SPLIT)),
                            stop=(c in (SPLIT - 1, NCH - 1)))

                with tc.tile_pool(name="psV", bufs=1, space="PSUM") as psV, \
                     tc.tile_pool(name="psX", bufs=1, space="PSUM") as psX:
                    x_ps = [psX.tile([128, 512], F32, tag=f"x{n}",
                                     name=f"x{n}")
                            for n in range(L_PAD // 512)]
                    def drain(xp_t, arin_t, arout_t):
                        for n in range(L_PAD // 512):
                            nc.scalar.copy(xp_t[:, 512 * n:512 * (n + 1)],
                                           x_ps[n][0:DH, :])
                            nc.sync.dma_start(
                                arin_t[:, 512 * n:512 * (n + 1)],
                                xp_t[:, 512 * n:512 * (n + 1)])
                        nc.gpsimd.collective_compute(
                            "ReduceScatter", ALU.add,
                            replica_groups=[list(range(NCORES))],
                            ins=[arin_t.opt()], outs=[arout_t.opt()])

                    xp_a = bpool.tile([DH, L_PAD], F32, tag="xpa")
                    xp_b = bpool.tile([DH, L_PAD], F32, tag="xpb")
                    for b in range(NPS):
                        w = min(512, PSHARD - 512 * b)
                        dk_t = dkpool.tile([128, KL * 512], BF16, tag="dk",
                                           name=f"dk{b}")
                        # 3D gather: for each k, the block's w columns
                        src_ap = p_d.ap().rearrange(
                            "p (k j) -> p k j", k=KL)[:, :, 512 * b:512 * b + w]
                        nc.sync.dma_start(
                            dk_t[:].rearrange("p (k j) -> p k j",
                                              k=KL)[:, :, 0:w], src_ap)
                        vf_ps = psV.tile([128, w], F32, tag="vf",
                                         name=f"vf{b}")
                        for half, lt in ((0, lh_all), (1, ll_all)):
                            for k in range(KL):
                                nc.tensor.matmul(
                                    vf_ps[:], lt[:, 128 * k:128 * (k + 1)],
                                    dk_t[:, 512 * k:512 * k + w],
                                    start=(k == 0 and half == 0),
                                    stop=(k == KL - 1 and half == 1))
                        nc.scalar.copy(ysb[:, 512 * b:512 * b + w],
                                       vf_ps[0:97, :])
                        for c in range(4 * b, min(4 * b + 4, NCH)):
                            chunk_softmax(c)
                            if c >= 1:
                                chunk_flow(c - 1)
                            # first-half flows complete -> overlap its
                            # ReduceScatter with the rest of the pipeline
                            if c == SPLIT:
                                drain(xp_a, ar_in_a, ar_out_a)
                    chunk_flow(NCH - 1)
                    drain(xp_b, ar_in_b, ar_out_b)

                # ---- BPR epilogue in the folded (96, 256) layout ----
                ib = cpool.tile([DH, 256], F32, tag="ib")
                nc.vector.reciprocal(ib[:], kb[:])
                bb2 = cpool.tile([DH, 256], F32, tag="bb2")
                nc.vector.tensor_scalar(bb2[:], bb[:], float(EPS), 4.0,
                                        ALU.max, ALU.min)
                ab = cpool.tile([DH, 256], F32, tag="ab")
                nc.scalar.activation(ab[:], lab[:], AF.Exp)
                atf = cpool.tile([DH, 256], F32, tag="atf")
                nc.vector.tensor_mul(atf[:], ab[:], ttf[:])
                xg_a = bpool.tile([DH, 256], F32, tag="xga")
                nc.sync.dma_start(
                    xg_a[:], ar_out_a.rearrange("d (a l) -> (d a) l", a=FB))
                xg_b = bpool.tile([DH, 256], F32, tag="xgb")
                nc.sync.dma_start(
                    xg_b[:], ar_out_b.rearrange("d (a l) -> (d a) l", a=FB))
                xg = bpool.tile([DH, 256], F32, tag="xg")
                nc.vector.tensor_add(xg[:], xg_a[:], xg_b[:])
                t0 = bpool.tile([DH, 256], F32, tag="t0")
                nc.vector.tensor_mul(t0[:], xg[:], ib[:])
                nc.vector.tensor_scalar_max(t0[:], t0[:], 1e-35)
                t1 = bpool.tile([DH, 256], F32, tag="t1")
                nc.scalar.activation(t1[:], t0[:], AF.Ln)
                nc.vector.tensor_mul(t1[:], t1[:], bb2[:])
                t2 = bpool.tile([DH, 256], F32, tag="t2")
                nc.scalar.activation(t2[:], t1[:], AF.Exp)
                nc.vector.tensor_mul(t2[:], t2[:], atf[:])
                o_t = bpool.tile([DH, 256], F32, tag="o")
                nc.vector.tensor_add(o_t[:], t2[:], ttf[:])
                nc.sync.dma_start(p_out.ap(), o_t[:])

    nc.compile()
    return nc


_CACHE = {}
LAST_RESULT = None


def _get_program(PSHARD, SLOT):
    key = (PSHARD, SLOT)
    if key not in _CACHE:
        _CACHE[key] = _build_program(PSHARD, SLOT)
    return _CACHE[key]


def _fold96(v_lpad):
    """(L_PAD,) per-link vector -> (96, 256) folded layout (row 8*d + a holds
    link block [256a, 256(a+1)) for every local day-hour d)."""
    return np.ascontiguousarray(
        np.tile(v_lpad.reshape(FB, 256), (DHS, 1)).astype(np.float32))


def kernel(X, theta_raw, theta_links, q_sqrt, log_alpha, beta_raw, k, D,
           od_of_path, n_ods):
    X = np.asarray(X, np.float32)
    D = np.asarray(D, np.float32)
    od = np.asarray(od_of_path, np.int32)
    assert X.shape == (ND, NH, NL, NF + 1) and D.shape == (NL, NP)
    assert int(n_ods) == NOD

    od_per_core = (NOD + NCORES - 1) // NCORES
    bounds = np.searchsorted(od, np.arange(0, NOD + 1, od_per_core)[:NCORES + 1])
    bounds[0], bounds[-1] = 0, NP
    cnts = np.diff(bounds)
    PSHARD = int(np.ceil(cnts.max() / 128) * 128)
    NCH = PSHARD // 128
    NPS = (PSHARD + 511) // 512

    max_span = 1
    for i in range(NCORES):
        odl = od[bounds[i]:bounds[i + 1]]
        for c in range(0, len(odl), 128):
            ch = odl[c:c + 128]
            if len(ch):
                max_span = max(max_span, int(ch[-1] - ch[0]) + 1)
    W = int(np.ceil(max_span / 32) * 32)
    SLOT = W
    SL1 = SLOT + 1

    nc = _get_program(PSHARD, SLOT)

    # ---- host-side shard construction (index bookkeeping + relayout only) --
    Xf = X.reshape(DH, NL, NF + 1)
    ttf_full = np.zeros((DH, L_PAD), np.float32)
    ttf_full[:, :NL] = Xf[:, :, 0]
    ft_h = np.zeros((DH, NF, L_PAD), np.float32)
    for f in range(NF):
        ft_h[:, f, :NL] = Xf[:, :, f + 1]
    ft_h = np.ascontiguousarray(ft_h.reshape(DH, NF * L_PAD))

    def padded_vec(v, fill=0.0):
        o = np.full(L_PAD, fill, np.float32)
        o[:NL] = v
        return o

    tl_h = np.ascontiguousarray(
        padded_vec(np.asarray(theta_links, np.float32)).reshape(KL, 128).T)
    kb_h = _fold96(padded_vec(np.asarray(k, np.float32), fill=1.0))
    bb_h = _fold96(padded_vec(np.asarray(beta_raw, np.float32)))
    lab_h = _fold96(padded_vec(np.asarray(log_alpha, np.float32)))
    th_h = np.asarray(theta_raw, np.float32).reshape(1, NF)
    qsr = np.asarray(q_sqrt, np.float32)
    id_h = np.eye(128, dtype=np.float32)

    in_maps = []
    for i in range(NCORES):
        lo, hi = bounds[i], bounds[i + 1]
        cnt = hi - lo
        odl = od[lo:hi]

        Dsh = np.zeros((L_PAD, PSHARD), np.float32)
        Dsh[:NL, :cnt] = D[:, lo:hi]
        # link-chunk-major D: dkb[p, PSHARD*k + j] = D[128k+p, j]
        dkb = np.ascontiguousarray(
            Dsh.reshape(KL, 128, PSHARD).transpose(1, 0, 2)
            .reshape(128, KL * PSHARD)).astype(ml_dtypes.bfloat16)
        dt_h = np.ascontiguousarray(Dsh.T).astype(
            ml_dtypes.bfloat16).reshape(NCH, 128, L_PAD)

        s_h = np.zeros((128, NCH, SL1), ml_dtypes.bfloat16)
        stm_h = np.zeros((SLOT, NCH, 128), ml_dtypes.bfloat16)
        stf_h = np.zeros((1, NCH, 128), ml_dtypes.bfloat16)
        stb_h = np.zeros((1, NCH, 128), ml_dtypes.bfloat16)
        qs_h = np.zeros(PSHARD, np.float32)
        qs_h[:cnt] = qsr[odl]
        qs_h = np.ascontiguousarray(qs_h.reshape(NCH, 128).T)

        firsts, lasts = {}, {}
        for c in range(NCH):
            ch = odl[128 * c:128 * (c + 1)]
            if len(ch):
                firsts[c], lasts[c] = int(ch[0]), int(ch[-1])
        for c in range(NCH):
            ch = odl[128 * c:128 * (c + 1)]
            if not len(ch):
                continue
            f0, l0 = firsts[c], lasts[c]
            asc = ch - f0
            rows = np.arange(len(ch))
            s_h[rows, c, asc] = 1.0
            s_h[rows[ch == l0], c, SLOT] = 1.0   # last-od partial row
            stm_h[asc, c, rows] = 1.0
            if c + 1 in firsts and firsts[c + 1] == l0:
                stf_h[0, c, rows[ch == l0]] = 1.0
            if c - 1 in lasts and lasts[c - 1] == f0:
                stb_h[0, c, rows[ch == f0]] = 1.0

        in_maps.append(dict(
            ft=ft_h, dkb=dkb, dtk=dt_h,
            seg=np.ascontiguousarray(s_h.reshape(128, NCH * SL1)),
            stm=np.ascontiguousarray(stm_h.reshape(SLOT, NCH * 128)),
            stf=np.ascontiguousarray(stf_h.reshape(1, NCH * 128)),
            stb=np.ascontiguousarray(stb_h.reshape(1, NCH * 128)),
            qsp=qs_h, th=th_h, tl=tl_h, idn=id_h,
            kb96=kb_h, bb96=bb_h, lab96=lab_h,
            ttf96=np.ascontiguousarray(
                ttf_full[DHS * i:DHS * (i + 1)].reshape(DH, 256))))

    trace = os.environ.get("BASS_KERNEL_TRACE", "0") == "1"
    global LAST_RESULT
    for _attempt in range(3):
        res = run_bass_kernel_spmd(nc, in_maps, core_ids=list(range(NCORES)),
                                   trace=trace)
        LAST_RESULT = res
        parts = [r["out"].reshape(DHS, L_PAD) for r in res.results]
        out = np.concatenate(parts, axis=0)[:, :NL]
        if np.isfinite(out).all():
            break
    return np.ascontiguousarray(out).reshape(ND, NH, NL).astype(np.float32)



# revision 4
# speedup vs baseline: 1.1829x; 1.1829x over previous
"""Trainium2 Bass kernel for the AESUELOGIT segment-reduce problem.

Strategy (8 NeuronCores, SPMD):
  - Shard the 20000 paths across cores ALIGNED TO OD BOUNDARIES (core i owns
    ods [500i, 500(i+1)) and all their paths).  The segmented softmax is then
    fully core-local: no denominator collective is needed.
  - v2 head: features arrive host-transposed (link-partition, bf16), so the
    link-utility lhsT is built by a handful of DVE ops, k-group pipelined with
    the feature DMA -- no tensor-engine transposes before the first matmul.
    The first D block's gather DMA is issued at t=0 on the other HWDGE ring.
  - matmul1 is a SINGLE bf16 pass (numerics sim: rel err 8.6e-5 vs 2e-2 gate).
    Per 128-path chunk: transpose -> exp (bias adds the theta_links row,
    written straight to bf16) -> segment-sum matmul -> denominator gather
    matmuls -> flows (stt straight to bf16) -> matmul2 accumulation over all
    chunks into 4 PSUM banks.
  - ONE bf16 ReduceScatter of the (96, 2048) partial link flows at the end;
    each core runs the BPR epilogue on its 12 day-hour rows in a folded
    (96, 256) layout; the host concatenates the 8 slices.
"""

import os

import numpy as np
import ml_dtypes

import concourse.bacc as bacc
import concourse.bass as bass
import concourse.mybir as mybir
import concourse.tile as tile
from concourse.bass_utils import run_bass_kernel_spmd

F32 = mybir.dt.float32
BF16 = mybir.dt.bfloat16
AF = mybir.ActivationFunctionType
ALU = mybir.AluOpType

ND, NH, NL, NF = 4, 24, 2000, 4
NP, NOD, NCORES = 20000, 4000, 8
DH = ND * NH            # 96
DHS = DH // NCORES      # 12 day-hour rows per core after ReduceScatter
L_PAD = 2048            # links padded to 16*128
KL = L_PAD // 128       # 16 link chunks
KG = 4                  # k-groups for the head pipeline (4 chunks each)
KPG = KL // KG
FB = L_PAD // 256       # 8 link blocks in the folded epilogue layout
EPS = 1e-12


def _build_program(PSHARD, SLOT):
    """Emit the SPMD Bass program (identical on all cores)."""
    NCH = PSHARD // 128
    NPS = (PSHARD + 511) // 512          # 512-path blocks
    SL1 = SLOT + 1                       # seg matmul emits asc slots + last-od
    nc = bacc.Bacc("TRN2", target_bir_lowering=False, debug=False,
                   num_devices=NCORES)

    # ---- parameters (per-core shards) ----
    # features transposed: [link%128, (kg, f, kk, dh)] bf16
    p_ft = nc.dram_tensor("ftt", [128, KG * NF * KPG * DH], BF16,
                          kind="ExternalInput")
    # D link-chunk-major without block padding:
    # dkb[p, PSHARD*k + j] = D[128k+p, j]
    p_d = nc.dram_tensor("dkb", [128, KL * PSHARD], BF16,
                         kind="ExternalInput")
    p_dt = nc.dram_tensor("dtk", [NCH, 128, L_PAD], BF16, kind="ExternalInput")
    p_s = nc.dram_tensor("seg", [128, NCH * SL1], BF16, kind="ExternalInput")
    p_stm = nc.dram_tensor("stm", [SLOT, NCH * 128], BF16,
                           kind="ExternalInput")
    p_stf = nc.dram_tensor("stf", [1, NCH * 128], BF16, kind="ExternalInput")
    p_stb = nc.dram_tensor("stb", [1, NCH * 128], BF16, kind="ExternalInput")
    p_qs = nc.dram_tensor("qsp", [128, NCH], F32, kind="ExternalInput")
    p_th = nc.dram_tensor("th", [1, NF], F32, kind="ExternalInput")
    p_tl = nc.dram_tensor("tl", [128, KL], F32, kind="ExternalInput")
    p_id = nc.dram_tensor("idn", [128, 128], F32, kind="ExternalInput")
    # folded (96, 256) epilogue tensors (host-replicated layouts, no math)
    p_kb = nc.dram_tensor("kb96", [DH, 256], F32, kind="ExternalInput")
    p_bb = nc.dram_tensor("bb96", [DH, 256], F32, kind="ExternalInput")
    p_lab = nc.dram_tensor("lab96", [DH, 256], F32, kind="ExternalInput")
    p_ttf = nc.dram_tensor("ttf96", [DH, 256], F32, kind="ExternalInput")
    p_out = nc.dram_tensor("out", [DH, 256], F32, kind="ExternalOutput")

    with tile.TileContext(nc) as tc:
        with tc.tile_pool(name="const", bufs=1) as cpool, \
             tc.tile_pool(name="dram", bufs=1, space="DRAM") as dpool, \
             tc.tile_pool(name="big", bufs=1) as bpool, \
             tc.tile_pool(name="stream", bufs=4) as spool, \
             tc.tile_pool(name="dkp", bufs=3) as dkpool, \
             tc.tile_pool(name="dtp", bufs=6) as dtpool:

            # critical-path loads: features (sync ring, kg-major so the lhsT
            # build pipelines), first D block gather (scalar ring)
            GW = NF * KPG * DH               # cols per k-group in p_ft
            fts = bpool.tile([128, KG * GW], BF16, tag="fts")
            for g in range(KG):
                nc.sync.dma_start(fts[:, g * GW:(g + 1) * GW],
                                  p_ft.ap()[:, g * GW:(g + 1) * GW])
            th_sb = cpool.tile([1, NF], F32, tag="th")
            nc.scalar.dma_start(th_sb[:], p_th.ap())
            tl_sb = cpool.tile([128, KL], F32, tag="tl")
            nc.scalar.dma_start(tl_sb[:], p_tl.ap())
            ident = cpool.tile([128, 128], F32, tag="ident")
            nc.scalar.dma_start(ident[:], p_id.ap())

            # D blocks: even on scalar, odd on sync (behind features)
            dk_t = [dkpool.tile([128, KL * 512], BF16, tag="dk",
                                name=f"dk{b}") for b in range(NPS)]

            def load_dk(b):
                w = min(512, PSHARD - 512 * b)
                src_ap = p_d.ap().rearrange(
                    "p (k j) -> p k j", k=KL)[:, :, 512 * b:512 * b + w]
                eng = nc.scalar if b % 2 == 0 else nc.sync
                eng.dma_start(
                    dk_t[b][:].rearrange("p (k j) -> p k j",
                                         k=KL)[:, :, 0:w], src_ap)

            load_dk(0)

            # chunk-op constants (scalar ring, behind dk0's issue but small)
            s_all = cpool.tile([128, NCH * SL1], BF16, tag="s_all")
            nc.scalar.dma_start(s_all[:], p_s.ap())
            stm_all = cpool.tile([SLOT, NCH * 128], BF16, tag="stm_all")
            nc.scalar.dma_start(stm_all[:], p_stm.ap())
            stf_all = cpool.tile([1, NCH * 128], BF16, tag="stf_all")
            nc.scalar.dma_start(stf_all[:], p_stf.ap())
            stb_all = cpool.tile([1, NCH * 128], BF16, tag="stb_all")
            nc.scalar.dma_start(stb_all[:], p_stb.ap())
            qs_sb = cpool.tile([128, NCH], F32, tag="qs")
            nc.scalar.dma_start(qs_sb[:], p_qs.ap())
            if NPS > 1:
                load_dk(1)

            # epilogue constants on the software ring (needed only at the end)
            kb = cpool.tile([DH, 256], F32, tag="kb")
            nc.gpsimd.dma_start(kb[:], p_kb.ap())
            bb = cpool.tile([DH, 256], F32, tag="bb")
            nc.gpsimd.dma_start(bb[:], p_bb.ap())
            lab = cpool.tile([DH, 256], F32, tag="lab")
            nc.gpsimd.dma_start(lab[:], p_lab.ap())
            ttf = cpool.tile([DH, 256], F32, tag="ttf")
            nc.gpsimd.dma_start(ttf[:], p_ttf.ap())

            thc = cpool.tile([1, NF], F32, tag="thc")
            nc.vector.tensor_scalar_min(thc[:], th_sb[:], 0.0)
            ones = cpool.tile([1, 128], F32, tag="ones")
            nc.vector.memset(ones[:], 1.0)

            with tc.tile_pool(name="psA", bufs=3, space="PSUM") as psA:
                thb_ps = psA.tile([128, NF], F32, tag="m")
                nc.tensor.matmul(thb_ps[:], ones[:], thc[:],
                                 start=True, stop=True)
                thb = cpool.tile([128, NF], F32, tag="thb")
                nc.scalar.copy(thb[:], thb_ps[:])

                # lhsT build, k-group pipelined with the feature DMA:
                # v0t[l, (k, dh)] = sum_f theta_f * featsT, then packed to
                # lh[l, (k, dh..+tl)] with theta_links in col 96 of each chunk
                v0t = bpool.tile([128, KL * DH], F32, tag="v0t")
                lh = bpool.tile([128, KL * 97], BF16, tag="lh")
                GV = KPG * DH                # v0t cols per k-group
                for g in range(KG):
                    base = g * GW
                    nc.vector.tensor_scalar_mul(
                        v0t[:, g * GV:(g + 1) * GV],
                        fts[:, base:base + GV], thb[:, 0:1])
                    for f in range(1, NF):
                        nc.vector.scalar_tensor_tensor(
                            v0t[:, g * GV:(g + 1) * GV],
                            fts[:, base + f * GV:base + (f + 1) * GV],
                            thb[:, f:f + 1],
                            v0t[:, g * GV:(g + 1) * GV], ALU.mult, ALU.add)
                    lh3 = lh[:].rearrange("p (k j) -> p k j", j=97)
                    nc.vector.tensor_copy(
                        lh3[:, g * KPG:(g + 1) * KPG, 0:DH],
                        v0t[:, g * GV:(g + 1) * GV].rearrange(
                            "p (k j) -> p k j", j=DH))
                    nc.vector.tensor_copy(
                        lh3[:, g * KPG:(g + 1) * KPG, DH:DH + 1],
                        tl_sb[:, g * KPG:(g + 1) * KPG].rearrange(
                            "p (k j) -> p k j", j=1))

                # ---- the block pipeline ----
                ysb = bpool.tile([97, PSHARD], F32, tag="ysb")
                evb = bpool.tile([128, DH * NCH], BF16, tag="evb")
                tall = bpool.tile([SL1, DH * NCH], BF16, tag="tall")
                ft_bf = bpool.tile([128, DH * NCH], BF16, tag="ftb")
                qsq = cpool.tile([128, NCH], F32, tag="qsq")
                nc.vector.tensor_mul(qsq[:], qs_sb[:], qs_sb[:])
                ar_in = dpool.tile([DH, L_PAD], BF16, tag="arin")
                ar_out = dpool.tile([DHS, L_PAD], BF16, tag="arout")

                def chunk_softmax(c):
                    """transpose -> exp -> segment sums for path chunk c."""
                    yt_ps = psA.tile([128, 97], F32, tag="m",
                                     name=f"yt{c}")
                    nc.tensor.matmul(yt_ps[:], ysb[:, 128 * c:128 * (c + 1)],
                                     ident[:97, :97], is_transpose=True,
                                     start=True, stop=True)
                    cvec = spool.tile([128, 1], F32, tag="cvec")
                    nc.scalar.copy(cvec[:], yt_ps[:, DH:DH + 1])
                    nc.scalar.activation(evb[:, DH * c:DH * (c + 1)],
                                         yt_ps[:, 0:DH], AF.Exp, bias=cvec[:])
                    ts_ps = psA.tile([SL1, DH], F32, tag="m",
                                     name=f"seg{c}")
                    nc.tensor.matmul(ts_ps[:],
                                     s_all[:, SL1 * c:SL1 * (c + 1)],
                                     evb[:, DH * c:DH * (c + 1)],
                                     start=True, stop=True)
                    nc.vector.tensor_copy(tall[:, DH * c:DH * (c + 1)],
                                          ts_ps[:])

                def chunk_flow(c):
                    """denominator gather + path flows + matmul2 for chunk c
                    (requires chunk c+1's segment sums, except the last)."""
                    g_ps = psA.tile([128, DH], F32, tag="m", name=f"g{c}")
                    cn = (c + 1) % NCH
                    cp = (c - 1) % NCH
                    nc.tensor.matmul(g_ps[:],
                                     stm_all[:, 128 * c:128 * (c + 1)],
                                     tall[0:SLOT, DH * c:DH * (c + 1)],
                                     start=True, stop=False)
                    nc.tensor.matmul(g_ps[:],
                                     stf_all[:, 128 * c:128 * (c + 1)],
                                     tall[0:1, DH * cn:DH * cn + DH],
                                     start=False, stop=False)
                    # row SLOT of chunk cp = its last-od partial; bounce it
                    # through a base-0 tile so the matmul base partitions match
                    tl_b = spool.tile([1, DH], BF16, tag="tlb")
                    nc.scalar.copy(tl_b[:],
                                   tall[SLOT:SL1, DH * cp:DH * cp + DH])
                    nc.tensor.matmul(g_ps[:],
                                     stb_all[:, 128 * c:128 * (c + 1)],
                                     tl_b[:], start=False, stop=True)
                    rec = spool.tile([128, DH], F32, tag="rec")
                    nc.vector.tensor_scalar_max(rec[:], g_ps[:], 1e-30)
                    nc.vector.reciprocal(rec[:], rec[:])
                    nc.vector.scalar_tensor_tensor(
                        ft_bf[:, DH * c:DH * (c + 1)],
                        evb[:, DH * c:DH * (c + 1)],
                        qsq[:, c:c + 1], rec[:], ALU.mult, ALU.mult)
                    for n in range(L_PAD // 512):
                        nc.tensor.matmul(
                            x_ps[n][:], ft_bf[:, DH * c:DH * (c + 1)],
                            dt_t[c][:, 512 * n:512 * (n + 1)],
                            start=(c == 0), stop=(c == NCH - 1))

                with tc.tile_pool(name="psV", bufs=1, space="PSUM") as psV, \
                     tc.tile_pool(name="psX", bufs=1, space="PSUM") as psX:
                    x_ps = [psX.tile([DH, 512], F32, tag=f"x{n}",
                                     name=f"x{n}")
                            for n in range(L_PAD // 512)]
                    dt_t = [dtpool.tile([128, L_PAD], BF16, tag="dt",
                                        name=f"dt{c}") for c in range(NCH)]

                    def load_dt(c):
                        eng = nc.gpsimd if c % 2 == 0 else nc.scalar
                        eng.dma_start(dt_t[c][:], p_dt.ap()[c])

                    load_dt(0)
                    load_dt(1)
                    for b in range(NPS):
                        w = min(512, PSHARD - 512 * b)
                        if b + 2 < NPS:
                            load_dk(b + 2)
                        vf_ps = psV.tile([97, w], F32, tag="vf",
                                         name=f"vf{b}")
                        for k in range(KL):
                            nc.tensor.matmul(
                                vf_ps[:], lh[:, 97 * k:97 * (k + 1)],
                                dk_t[b][:, 512 * k:512 * k + w],
                                start=(k == 0), stop=(k == KL - 1))
                        nc.scalar.copy(ysb[:, 512 * b:512 * b + w],
                                       vf_ps[:])
                        for c in range(4 * b, min(4 * b + 4, NCH)):
                            if c + 2 < NCH:
                                load_dt(c + 2)
                            chunk_softmax(c)
                            if c >= 1:
                                chunk_flow(c - 1)
                    chunk_flow(NCH - 1)

                    # drain: PSUM -> bf16 -> DRAM -> one ReduceScatter
                    xb = bpool.tile([DH, L_PAD], BF16, tag="xb")
                    for n in range(L_PAD // 512):
                        if n % 2 == 0:
                            nc.scalar.copy(xb[:, 512 * n:512 * (n + 1)],
                                           x_ps[n][:])
                        else:
                            nc.vector.tensor_copy(
                                xb[:, 512 * n:512 * (n + 1)], x_ps[n][:])
                        nc.sync.dma_start(
                            ar_in[:, 512 * n:512 * (n + 1)],
                            xb[:, 512 * n:512 * (n + 1)])
                    nc.gpsimd.collective_compute(
                        "ReduceScatter", ALU.add,
                        replica_groups=[list(range(NCORES))],
                        ins=[ar_in.opt()], outs=[ar_out.opt()])

                # ---- BPR epilogue in the folded (96, 256) layout ----
                ib = cpool.tile([DH, 256], F32, tag="ib")
                nc.vector.reciprocal(ib[:], kb[:])
                bb2 = cpool.tile([DH, 256], F32, tag="bb2")
                nc.vector.tensor_scalar(bb2[:], bb[:], float(EPS), 4.0,
                                        ALU.max, ALU.min)
                ab = cpool.tile([DH, 256], F32, tag="ab")
                nc.scalar.activation(ab[:], lab[:], AF.Exp)
                atf = cpool.tile([DH, 256], F32, tag="atf")
                nc.vector.tensor_mul(atf[:], ab[:], ttf[:])
                xg = bpool.tile([DH, 256], BF16, tag="xg")
                nc.sync.dma_start(
                    xg[:], ar_out.rearrange("d (a l) -> (d a) l", a=FB))
                t0 = bpool.tile([DH, 256], F32, tag="t0")
                nc.vector.tensor_mul(t0[:], xg[:], ib[:])
                nc.vector.tensor_scalar_max(t0[:], t0[:], 1e-35)
                t1 = bpool.tile([DH, 256], F32, tag="t1")
                nc.scalar.activation(t1[:], t0[:], AF.Ln)
                nc.vector.tensor_mul(t1[:], t1[:], bb2[:])
                t2 = bpool.tile([DH, 256], F32, tag="t2")
                nc.scalar.activation(t2[:], t1[:], AF.Exp)
                nc.vector.tensor_mul(t2[:], t2[:], atf[:])
                o_t = bpool.tile([DH, 256], F32, tag="o")
                nc.vector.tensor_add(o_t[:], t2[:], ttf[:])
                nc.sync.dma_start(p_out.ap(), o_t[:])

    nc.compile()
    return nc


_CACHE = {}
LAST_RESULT = None


def _get_program(PSHARD, SLOT):
    key = (PSHARD, SLOT)
    if key not in _CACHE:
        _CACHE[key] = _build_program(PSHARD, SLOT)
    return _CACHE[key]


def _fold96(v_lpad):
    """(L_PAD,) per-link vector -> (96, 256) folded layout (row 8*d + a holds
    link block [256a, 256(a+1)) for every local day-hour d)."""
    return np.ascontiguousarray(
        np.tile(v_lpad.reshape(FB, 256), (DHS, 1)).astype(np.float32))


def kernel(X, theta_raw, theta_links, q_sqrt, log_alpha, beta_raw, k, D,
           od_of_path, n_ods):
    X = np.asarray(X, np.float32)
    D = np.asarray(D, np.float32)
    od = np.asarray(od_of_path, np.int32)
    assert X.shape == (ND, NH, NL, NF + 1) and D.shape == (NL, NP)
    assert int(n_ods) == NOD

    od_per_core = (NOD + NCORES - 1) // NCORES
    bounds = np.searchsorted(od, np.arange(0, NOD + 1, od_per_core)[:NCORES + 1])
    bounds[0], bounds[-1] = 0, NP
    cnts = np.diff(bounds)
    PSHARD = int(np.ceil(cnts.max() / 128) * 128)
    NCH = PSHARD // 128
    NPS = (PSHARD + 511) // 512

    max_span = 1
    for i in range(NCORES):
        odl = od[bounds[i]:bounds[i + 1]]
        for c in range(0, len(odl), 128):
            ch = odl[c:c + 128]
            if len(ch):
                max_span = max(max_span, int(ch[-1] - ch[0]) + 1)
    W = int(np.ceil(max_span / 32) * 32)
    SLOT = W
    SL1 = SLOT + 1

    nc = _get_program(PSHARD, SLOT)

    # ---- host-side shard construction (index bookkeeping + relayout only) --
    Xf = X.reshape(DH, NL, NF + 1)
    ttf_full = np.zeros((DH, L_PAD), np.float32)
    ttf_full[:, :NL] = Xf[:, :, 0]
    # featsT[link%128, (kg, f, kk, dh)] bf16
    ftt = np.zeros((L_PAD, NF, DH), np.float32)
    for f in range(NF):
        ftt[:NL, f, :] = Xf[:, :, f + 1].T
    ftt = (ftt.reshape(KG, KPG, 128, NF, DH).transpose(2, 0, 3, 1, 4)
           .reshape(128, KG * NF * KPG * DH))
    ftt_h = np.ascontiguousarray(ftt).astype(ml_dtypes.bfloat16)

    def padded_vec(v, fill=0.0):
        o = np.full(L_PAD, fill, np.float32)
        o[:NL] = v
        return o

    tl_h = np.ascontiguousarray(
        padded_vec(np.asarray(theta_links, np.float32)).reshape(KL, 128).T)
    kb_h = _fold96(padded_vec(np.asarray(k, np.float32), fill=1.0))
    bb_h = _fold96(padded_vec(np.asarray(beta_raw, np.float32)))
    lab_h = _fold96(padded_vec(np.asarray(log_alpha, np.float32)))
    th_h = np.asarray(theta_raw, np.float32).reshape(1, NF)
    qsr = np.asarray(q_sqrt, np.float32)
    id_h = np.eye(128, dtype=np.float32)

    in_maps = []
    for i in range(NCORES):
        lo, hi = bounds[i], bounds[i + 1]
        cnt = hi - lo
        odl = od[lo:hi]

        Dsh = np.zeros((L_PAD, PSHARD), np.float32)
        Dsh[:NL, :cnt] = D[:, lo:hi]
        # link-chunk-major D: dkb[p, PSHARD*k + j] = D[128k+p, j]
        dkb = np.ascontiguousarray(
            Dsh.reshape(KL, 128, PSHARD).transpose(1, 0, 2)
            .reshape(128, KL * PSHARD)).astype(ml_dtypes.bfloat16)
        dt_h = np.ascontiguousarray(Dsh.T).astype(
            ml_dtypes.bfloat16).reshape(NCH, 128, L_PAD)

        s_h = np.zeros((128, NCH, SL1), ml_dtypes.bfloat16)
        stm_h = np.zeros((SLOT, NCH, 128), ml_dtypes.bfloat16)
        stf_h = np.zeros((1, NCH, 128), ml_dtypes.bfloat16)
        stb_h = np.zeros((1, NCH, 128), ml_dtypes.bfloat16)
        qs_h = np.zeros(PSHARD, np.float32)
        qs_h[:cnt] = qsr[odl]
        qs_h = np.ascontiguousarray(qs_h.reshape(NCH, 128).T)

        firsts, lasts = {}, {}
        for c in range(NCH):
            ch = odl[128 * c:128 * (c + 1)]
            if len(ch):
                firsts[c], lasts[c] = int(ch[0]), int(ch[-1])
        for c in range(NCH):
            ch = odl[128 * c:128 * (c + 1)]
            if not len(ch):
                continue
            f0, l0 = firsts[c], lasts[c]
            asc = ch - f0
            rows = np.arange(len(ch))
            s_h[rows, c, asc] = 1.0
            s_h[rows[ch == l0], c, SLOT] = 1.0   # last-od partial row
            stm_h[asc, c, rows] = 1.0
            if c + 1 in firsts and firsts[c + 1] == l0:
                stf_h[0, c, rows[ch == l0]] = 1.0
            if c - 1 in lasts and lasts[c - 1] == f0:
                stb_h[0, c, rows[ch == f0]] = 1.0

        in_maps.append(dict(
            ftt=ftt_h, dkb=dkb, dtk=dt_h,
            seg=np.ascontiguousarray(s_h.reshape(128, NCH * SL1)),
            stm=np.ascontiguousarray(stm_h.reshape(SLOT, NCH * 128)),
            stf=np.ascontiguousarray(stf_h.reshape(1, NCH * 128)),
            stb=np.ascontiguousarray(stb_h.reshape(1, NCH * 128)),
            qsp=qs_h, th=th_h, tl=tl_h, idn=id_h,
            kb96=kb_h, bb96=bb_h, lab96=lab_h,
            ttf96=np.ascontiguousarray(
                ttf_full[DHS * i:DHS * (i + 1)].reshape(DH, 256))))

    trace = os.environ.get("BASS_KERNEL_TRACE", "0") == "1"
    global LAST_RESULT
    for _attempt in range(3):
        res = run_bass_kernel_spmd(nc, in_maps, core_ids=list(range(NCORES)),
                                   trace=trace)
        LAST_RESULT = res
        parts = [r["out"].reshape(DHS, L_PAD) for r in res.results]
        out = np.concatenate(parts, axis=0)[:, :NL]
        if np.isfinite(out).all():
            break
    return np.ascontiguousarray(out).reshape(ND, NH, NL).astype(np.float32)
